# revision 55
# baseline (speedup 1.0000x reference)
"""TRN2 Bass kernel for nn_KnnModule (retrieval_knn).

Strategy (sharded over the 8 NeuronCores):
  - train set (100000 x 1024) is padded to 100352 rows and split into 8
    chunks of 12544; each core computes sims = features @ chunk.T
    (2048 x 12544) on the PE in fp8e4m3 with DoubleRow double-pumping
    (2 fp8 MACs/cell/cycle -> ~2x the fp32r/bf16 matmul rate, ~157 TF/s
    per core; the PE runs gap-free at ~100% of that roofline).
  - the (128, 512) fp32 PSUM sim tiles are staged to SBUF as fp16 by the
    Act engine; the DVE folds each pair of tiles (1024 cols) with a
    3-level tensor_tensor max tree into 128 group-maxima (groups of 8
    columns, stride 128), accumulating a per-row vector of 1664 fp16
    group maxima (12 tile pairs + a leading 256-wide odd half-tile that
    yields 128 groups of 2).
  - the DVE extracts top-8 (value, index) group candidates per row from
    three column parts of that vector (InstMax/InstMaxIndex), staggered
    across the tile-pair loop so the extraction never stalls the PE;
    every row leaves the device as 24 (fp16 value, uint16 group) pairs
    per core.
  - host merges the 8x24 group candidates per row, exactly rescores the
    members of the top NG=32 groups (256 candidate columns) in fp32, and
    reproduces the reference softmax voting for k in (10, 20, 100, 200).

  Why this is exact: with T=0.07 softmax weights are exactly 0.0 in fp32
  unless the sim is within ~7.4 of the row max.  Group maxima upper-bound
  their members, fp8 sim error is <~7 absolute (measured 5sigma ~ 6.2)
  and fp16 group quantization <~0.13, so every needed column lives in a
  group whose fp16 value ranks <= ~11 globally (measured <= 11 over 512
  rows); NG=32 rescored groups leaves 3x margin.  Rows where the margins
  could be violated (NG-th group within 16 of the max, a duplicated
  group index near the top from an fp16 tie, or a part's 8th slot near
  the top) are detected and recomputed exactly on the host (~1% of
  rows, batched into one GEMM).
"""

import numpy as np
import ml_dtypes

KS = (10, 20, 100, 200)
T = 0.07
NUM_CLASSES = 1000
B, N, D = 2048, 100000, 1024
NCORES = 8
NCHUNK = 12544  # per-core padded chunk (8*12544 = 100352 = 100000 + 352 pad)
TILE_N = 512
NPAIR = 12           # tile pairs folded into groups of 8
NGROUP = 13 * 128    # 1664 groups per (row, core-chunk)
# gmax col parts with separate top-8 extraction, staggered to keep the
# DVE off the critical path: A = odd tile + pairs 0..6, B = pairs 7..9,
# C = pairs 10..11
PARTS = ((0, 1024), (1024, 1408), (1408, 1664))
P = 128
MB = B // P
KO = D // P
NG = 32              # groups exactly rescored per row
MARGIN = 16.0        # host fallback margin (window 7.4 + fp8 + fp16 err)

F8_DT = ml_dtypes.float8_e4m3

_NC_CACHE = {}
_DEBUG_STATS = {}


def _build_bass():
    import concourse.bacc as bacc
    import concourse.mybir as mybir
    import concourse.tile as tile

    f8 = mybir.dt.float8e4
    f16 = mybir.dt.float16
    f32 = mybir.dt.float32
    u16 = mybir.dt.uint16
    DR = mybir.MatmulPerfMode.DoubleRow
    MAX = mybir.AluOpType.max

    nc = bacc.Bacc(
        "TRN2",
        target_bir_lowering=False,
        debug=False,
        enable_asserts=False,
    )
    # both inputs are pre-swizzled on the host into SBUF layout so every
    # DMA segment is one contiguous 2-8 KiB run per partition.  feat comes
    # in 5 chunks of (2, 2, 4, 4, 4) m-blocks so the first matmul only
    # waits on a 256-column transfer
    featT = nc.dram_tensor("featT", (D * B,), f8, kind="ExternalInput")
    trainT = nc.dram_tensor("trainT", (D * NCHUNK,), f8, kind="ExternalInput")
    out_all = nc.dram_tensor("gout", (B, 48), u16, kind="ExternalOutput")

    HT = 256
    ODD_SZ = P * KO * HT

    def dram_seg(t, off, n):
        return t.ap()[off : off + P * KO * n].rearrange(
            "(p ko n) -> p ko n", p=P, ko=KO
        )

    def train_seg(off, n):
        return dram_seg(trainT, off, n)

    with tile.TileContext(nc) as tc:
        with (
            tc.tile_pool(name="const", bufs=1) as cpool,
            tc.tile_pool(name="stream", bufs=2) as spool,
            tc.tile_pool(name="acc", bufs=1) as apool,
            tc.tile_pool(name="stage", bufs=4) as xpool,
            tc.tile_pool(name="fold", bufs=2) as fpool,
            tc.tile_pool(name="psum", bufs=8, space="PSUM") as ppool,
        ):
            # feat chunks by m-block: (2, 2, 4, 4, 4)
            FM = (2, 2, 4, 4, 4)
            FM0 = (0, 2, 4, 8, 12)
            feat_sb = [
                cpool.tile([P, KO, nm * P], f8, name=f"feat_{c}", tag=f"f{c}")
                for c, nm in enumerate(FM)
            ]

            def feat_dma(c):
                nc.sync.dma_start(
                    feat_sb[c], dram_seg(featT, P * KO * (FM0[c] * P), FM[c] * P)
                )

            def feat_slice(kop, m):
                c = next(i for i in range(4, -1, -1) if FM0[i] <= m)
                r = (m - FM0[c]) * P
                return feat_sb[c][:, 2 * kop : 2 * kop + 2, r : r + P]

            # group-maxima and output tiles hold 4 m-blocks each: all their
            # readers/writers are DVE instructions (engine-FIFO ordered), so
            # coarse tags cost nothing and save epilogue semaphore resets
            gmax_sb = [
                apool.tile([P, 4 * NGROUP], f16, name=f"gmax_{g}", tag=f"gm{g}")
                for g in range(MB // 4)
            ]
            # per m: cols 0..23 top-8 values (fp16 via bitcast), 24..47 idx
            out_sb = [
                apool.tile([P, 4, 48], u16, name=f"out_{g}", tag=f"o{g}")
                for g in range(MB // 4)
            ]

            def gmax(m, lo, hi):
                return gmax_sb[m // 4][:, (m % 4) * NGROUP + lo : (m % 4) * NGROUP + hi]

            def oslot(m, lo, hi):
                return out_sb[m // 4][:, m % 4, lo:hi]

            def top8(m, lo, hi, slot):
                vs = oslot(m, slot, slot + 8).bitcast(f16)
                nc.vector.max(out=vs, in_=gmax(m, lo, hi))
                nc.vector.max_index(
                    out=oslot(m, 24 + slot, 24 + slot + 8),
                    in_max=vs,
                    in_values=gmax(m, lo, hi),
                )

            # odd half-tile first (chunk cols 12288:12544 -> gmax[:, 0:128],
            # 128 groups of 2: cols 12288 + j + 128k, k<2).  DMA issue
            # order front-loads the first compute dependencies: odd train
            # tile, feat chunks 0-2, pair-0 train, feat chunks 3-4.
            trodd_sb = spool.tile([P, KO, HT], f8, name="trodd_sb", tag="trodd")
            nc.sync.dma_start(trodd_sb, train_seg(0, HT))
            feat_dma(0)
            feat_dma(1)
            feat_dma(2)
            tr0_sb = spool.tile([P, KO, 2 * TILE_N], f8, name="tr_sb", tag="train")
            nc.sync.dma_start(tr0_sb, train_seg(ODD_SZ, 1024))
            feat_dma(3)
            feat_dma(4)

            # warm-up: run throwaway matmuls on uninitialized SBUF while the
            # input DMAs are in flight, so the PE's HAM duty cycle reaches
            # 8/8 before real data arrives (first real MMs otherwise run ~2x
            # slow for ~3us)
            dum = cpool.tile([P, 2, TILE_N], f8, name="warm", tag="warm")
            nc.vector.memset(dum, 0)
            psw = ppool.tile([P, TILE_N], f32, name="ps", tag="ps")
            for _ in range(7):
                nc.tensor.matmul(
                    psw,
                    lhsT=dum[:, :, :P],
                    rhs=dum,
                    start=True,
                    stop=True,
                    perf_mode=DR,
                )
            for m in range(MB):
                ps = ppool.tile([P, TILE_N], f32, name="ps", tag="ps")
                for kop in range(KO // 2):
                    nc.tensor.matmul(
                        ps[:, :HT],
                        lhsT=feat_slice(kop, m),
                        rhs=trodd_sb[:, 2 * kop : 2 * kop + 2, :],
                        start=(kop == 0),
                        stop=(kop == KO // 2 - 1),
                        perf_mode=DR,
                    )
                s16 = xpool.tile([P, 2 * TILE_N], f16, name="s16", tag="s16")
                nc.scalar.copy(out=s16[:, :HT], in_=ps[:, :HT])
                nc.vector.tensor_tensor(
                    gmax(m, 0, P), s16[:, :128], s16[:, 128:HT], MAX
                )

            # 12 tile pairs (chunk cols 1024p : 1024p+1024 ->
            # gmax[:, 128+128p : 256+128p], groups of 8: 1024p + j + 128k)
            for p in range(NPAIR):
                if p == 0:
                    tr_sb = tr0_sb
                else:
                    tr_sb = spool.tile(
                        [P, KO, 2 * TILE_N], f8, name="tr_sb", tag="train"
                    )
                    nc.sync.dma_start(
                        tr_sb, train_seg(ODD_SZ + p * P * KO * 1024, 1024)
                    )
                for m in range(MB):
                    psA = ppool.tile([P, TILE_N], f32, name="ps", tag="ps")
                    psB = ppool.tile([P, TILE_N], f32, name="ps", tag="ps")
                    for kop in range(KO // 2):
                        for ps, col in ((psA, 0), (psB, TILE_N)):
                            nc.tensor.matmul(
                                ps,
                                lhsT=feat_slice(kop, m),
                                rhs=tr_sb[:, 2 * kop : 2 * kop + 2, col : col + TILE_N],
                                start=(kop == 0),
                                stop=(kop == KO // 2 - 1),
                                perf_mode=DR,
                            )
                    s16 = xpool.tile([P, 2 * TILE_N], f16, name="s16", tag="s16")
                    nc.scalar.copy(out=s16[:, :TILE_N], in_=psA)
                    nc.scalar.copy(out=s16[:, TILE_N:], in_=psB)
                    fa = fpool.tile([P, TILE_N], f16, name="fh", tag="fh")
                    nc.vector.tensor_tensor(fa, s16[:, :TILE_N], s16[:, TILE_N:], MAX)
                    fb = fpool.tile([P, 256], f16, name="fa", tag="fa")
                    nc.vector.tensor_tensor(fb, fa[:, :256], fa[:, 256:], MAX)
                    off = P + p * P
                    nc.vector.tensor_tensor(
                        gmax(m, off, off + P), fb[:, :128], fb[:, 128:], MAX
                    )
                    # staggered top-8 extraction:
                    #   part A (ready after pair 6): spread over pairs 7..9
                    #   part B (ready after pair 9): all at pair 10
                    #   part C (ready after pair 11): per-m at pair 11
                    if (
                        (p == 7 and m < 6)
                        or (p == 8 and 6 <= m < 11)
                        or (p == 9 and 11 <= m)
                    ):
                        top8(m, *PARTS[0], 0)
                    if p == 10:
                        top8(m, *PARTS[1], 8)
                    if p == NPAIR - 1:
                        top8(m, *PARTS[2], 16)

            oo = out_all.ap().rearrange("(g j p) c -> g p j c", p=P, j=4)
            for g in range(MB // 4):
                nc.sync.dma_start(oo[g], out_sb[g])

    nc.compile()
    return nc


def _get_nc():
    if "nc" not in _NC_CACHE:
        _NC_CACHE["nc"] = _build_bass()
    return _NC_CACHE["nc"]


def _group_cols():
    """(NGROUP, 8) chunk-column members per group; -1 = unused slot."""
    cols = np.full((NGROUP, 8), -1, np.int64)
    j = np.arange(128)
    k = np.arange(8)
    for p in range(NPAIR):
        cols[P + p * P : P + (p + 1) * P] = 1024 * p + j[:, None] + 128 * k[None, :]
    cols[0:P, :2] = 12288 + j[:, None] + 128 * np.arange(2)[None, :]
    return cols


_COLS = _group_cols()


def _vote(topv, labels):
    """Reproduce the reference's softmax voting given sorted top sims."""
    x = (topv / np.float32(T)).astype(np.float32)
    e = np.exp(x - x[:, :1], dtype=np.float32)
    s = e.sum(axis=1, keepdims=True, dtype=np.float32)
    w = (e / s).astype(np.float32)
    rows = np.broadcast_to(np.arange(topv.shape[0])[:, None], labels.shape)
    outs = []
    for k in KS:
        p = np.zeros((topv.shape[0], NUM_CLASSES), np.float32)
        np.add.at(p, (rows[:, :k], labels[:, :k]), w[:, :k])
        outs.append(p)
    return outs


def _exact_rows(F, TR, LB, rows, outs):
    """Batched exact recompute of the given rows (reference semantics)."""
    if len(rows) == 0:
        return
    s = (F[rows] @ TR.T).astype(np.float32)
    o = np.argsort(-s.astype(np.float64), axis=1, kind="stable")[:, :200]
    topv = np.take_along_axis(s, o, axis=1).astype(np.float32)
    labs = LB[o].astype(np.int64)
    sub = _vote(topv, labs)
    for i in range(len(KS)):
        outs[i][rows] = sub[i]


def _combine(F, TR, LB, vals, idxs):
    """vals/idxs: (NCORES, B, 24) fp32 / int64 device candidates."""
    # global group ids: slots 0..7 part A, 8..15 part B, 16..23 part C
    base = np.repeat([PARTS[0][0], PARTS[1][0], PARTS[2][0]], 8)
    gl = idxs + base[None, None, :]
    gl = gl + np.arange(NCORES)[:, None, None] * NGROUP
    v = vals.transpose(1, 0, 2).reshape(B, NCORES * 24).astype(np.float32)
    g = gl.transpose(1, 0, 2).reshape(B, NCORES * 24)

    order = np.argsort(-v, axis=1)
    v_s = np.take_along_axis(v, order, axis=1)
    g_s = np.take_along_axis(g, order, axis=1)
    amax = v_s[:, 0]

    # triggers
    trig = v_s[:, NG - 1] >= amax - MARGIN  # NG-th group near window
    # per-(core,part) duplicated group index near the top (fp16 tie)
    iv = idxs.transpose(1, 0, 2).reshape(B, NCORES, 3, 8)
    vv = vals.transpose(1, 0, 2).reshape(B, NCORES, 3, 8).astype(np.float32)
    si = np.sort(iv, axis=3)
    dup = (np.diff(si, axis=3) == 0).any(axis=3) & (
        vv.max(axis=3) >= amax[:, None, None] - MARGIN
    )
    trig |= dup.any(axis=(1, 2))
    # a part's 8th slot near the top (its 9th group may be within window)
    trig |= (vv[:, :, :, 7] >= amax[:, None, None] - MARGIN).any(axis=(1, 2))
    _DEBUG_STATS["trigger_rows"] = int(trig.sum())

    # exact rescore of the top NG groups' member columns
    top_g = g_s[:, :NG]
    core = top_g // NGROUP
    mem = _COLS[top_g % NGROUP]  # (B, NG, 8) chunk cols, -1 pad
    gcol = mem + core[:, :, None] * NCHUNK
    valid = (mem >= 0) & (gcol < N)
    gflat = np.where(valid, gcol, 0).reshape(B, NG * 8)

    exact = np.empty((B, NG * 8), np.float32)
    step = 256
    for b0 in range(0, B, step):
        b1 = min(b0 + step, B)
        exact[b0:b1] = np.einsum(
            "bkd,bd->bk", TR[gflat[b0:b1]], F[b0:b1], optimize=True
        )
    exact = np.where(valid.reshape(B, NG * 8), exact, -np.inf)

    # sort by exact value desc, ties by train index asc (lax.top_k order)
    ordk = np.lexsort((gflat, -exact.astype(np.float64)), axis=1)
    exact_s = np.take_along_axis(exact, ordk, axis=1)[:, :200].astype(np.float32)
    col_s = np.take_along_axis(gflat, ordk, axis=1)[:, :200]

    labels = np.where(exact_s > -np.inf, LB[col_s], 0).astype(np.int64)
    outs = _vote(exact_s, labels)

    _exact_rows(F, TR, LB, np.where(trig)[0], outs)
    return tuple(outs)


def _swz(seg8):
    """(ncols, D) fp8 rows -> device layout [p][ko][n], flattened."""
    return np.ascontiguousarray(
        seg8.T.reshape(KO, P, seg8.shape[0]).transpose(1, 0, 2)
    ).ravel()


def make_in_maps(F, TR):
    TRp = np.zeros((NCORES * NCHUNK, D), F8_DT)
    TRp[:N] = TR.astype(F8_DT)
    F8 = F.astype(F8_DT)
    fb = [(0, 256), (256, 512), (512, 1024), (1024, 1536), (1536, 2048)]
    feat8 = np.concatenate([_swz(F8[a:b]) for a, b in fb])
    in_maps = []
    for c in range(NCORES):
        TRc = TRp[c * NCHUNK : (c + 1) * NCHUNK]
        segs = [_swz(TRc[NPAIR * 1024 : NCHUNK])]
        segs += [_swz(TRc[p * 1024 : (p + 1) * 1024]) for p in range(NPAIR)]
        in_maps.append({"featT": feat8, "trainT": np.concatenate(segs)})
    return in_maps


def kernel(features_rank, train_features, train_labels):
    from concourse.bass_utils import run_bass_kernel_spmd

    F = np.ascontiguousarray(np.asarray(features_rank, dtype=np.float32))
    TR = np.ascontiguousarray(np.asarray(train_features, dtype=np.float32))
    LB = np.asarray(train_labels)

    nc = _get_nc()
    res = run_bass_kernel_spmd(nc, make_in_maps(F, TR), core_ids=list(range(NCORES)))

    raw = np.stack([np.asarray(res.results[c]["gout"]) for c in range(NCORES)])
    vals = raw[:, :, :24].view(np.float16).astype(np.float32)
    idxs = raw[:, :, 24:].astype(np.int64)
    return _combine(F, TR, LB, vals, idxs)


# revision 56
# speedup vs baseline: 1.0066x; 1.0066x over previous
"""TRN2 Bass kernel for nn_KnnModule (retrieval_knn).

Strategy (sharded over the 8 NeuronCores):
  - train set (100000 x 1024) is padded to 100352 rows and split into 8
    chunks of 12544; each core computes sims = features @ chunk.T
    (2048 x 12544) on the PE in fp8e4m3 with DoubleRow double-pumping
    (2 fp8 MACs/cell/cycle -> ~2x the fp32r/bf16 matmul rate, ~157 TF/s
    per core; the PE runs gap-free at ~100% of that roofline).
  - the (128, 512) fp32 PSUM sim tiles are staged to SBUF as fp16 by the
    Act engine; the DVE folds each pair of tiles (1024 cols) with a
    3-level tensor_tensor max tree into 128 group-maxima (groups of 8
    columns, stride 128), accumulating a per-row vector of 1664 fp16
    group maxima (12 tile pairs + a leading 256-wide odd half-tile that
    yields 128 groups of 2).
  - the DVE extracts top-8 (value, index) group candidates per row from
    three column parts of that vector (InstMax/InstMaxIndex), staggered
    across the tile-pair loop so the extraction never stalls the PE;
    every row leaves the device as 24 (fp16 value, uint16 group) pairs
    per core.
  - host merges the 8x24 group candidates per row, exactly rescores the
    members of the top NG=32 groups (256 candidate columns) in fp32, and
    reproduces the reference softmax voting for k in (10, 20, 100, 200).

  Why this is exact: with T=0.07 softmax weights are exactly 0.0 in fp32
  unless the sim is within ~7.4 of the row max.  Group maxima upper-bound
  their members, fp8 sim error is <~7 absolute (measured 5sigma ~ 6.2)
  and fp16 group quantization <~0.13, so every needed column lives in a
  group whose fp16 value ranks <= ~11 globally (measured <= 11 over 512
  rows); NG=32 rescored groups leaves 3x margin.  Rows where the margins
  could be violated (NG-th group within 16 of the max, a duplicated
  group index near the top from an fp16 tie, or a part's 8th slot near
  the top) are detected and recomputed exactly on the host (~1% of
  rows, batched into one GEMM).
"""

import numpy as np
import ml_dtypes

KS = (10, 20, 100, 200)
T = 0.07
NUM_CLASSES = 1000
B, N, D = 2048, 100000, 1024
NCORES = 8
NCHUNK = 12544  # per-core padded chunk (8*12544 = 100352 = 100000 + 352 pad)
TILE_N = 512
NPAIR = 12           # tile pairs folded into groups of 8
NGROUP = 13 * 128    # 1664 groups per (row, core-chunk)
# gmax col parts with separate top-8 extraction, staggered to keep the
# DVE off the critical path: A = odd tile + pairs 0..6, B = pairs 7..9,
# C = pairs 10..11
PARTS = ((0, 1024), (1024, 1408), (1408, 1664))
P = 128
MB = B // P
KO = D // P
NG = 32              # groups exactly rescored per row
MARGIN = 16.0        # host fallback margin (window 7.4 + fp8 + fp16 err)

F8_DT = ml_dtypes.float8_e4m3

_NC_CACHE = {}
_DEBUG_STATS = {}


def _build_bass():
    import concourse.bacc as bacc
    import concourse.mybir as mybir
    import concourse.tile as tile

    f8 = mybir.dt.float8e4
    f16 = mybir.dt.float16
    f32 = mybir.dt.float32
    u16 = mybir.dt.uint16
    DR = mybir.MatmulPerfMode.DoubleRow
    MAX = mybir.AluOpType.max

    nc = bacc.Bacc(
        "TRN2",
        target_bir_lowering=False,
        debug=False,
        enable_asserts=False,
    )
    # both inputs are pre-swizzled on the host into SBUF layout so every
    # DMA segment is one contiguous 2-8 KiB run per partition.  feat comes
    # in 5 chunks of (2, 2, 4, 4, 4) m-blocks so the first matmul only
    # waits on a 256-column transfer
    featT = nc.dram_tensor("featT", (D * B,), f8, kind="ExternalInput")
    trainT = nc.dram_tensor("trainT", (D * NCHUNK,), f8, kind="ExternalInput")
    out_all = nc.dram_tensor("gout", (B, 48), u16, kind="ExternalOutput")

    HT = 256
    ODD_SZ = P * KO * HT

    def dram_seg(t, off, n):
        return t.ap()[off : off + P * KO * n].rearrange(
            "(p ko n) -> p ko n", p=P, ko=KO
        )

    def train_seg(off, n):
        return dram_seg(trainT, off, n)

    with tile.TileContext(nc) as tc:
        with (
            tc.tile_pool(name="const", bufs=1) as cpool,
            tc.tile_pool(name="stream", bufs=2) as spool,
            tc.tile_pool(name="acc", bufs=1) as apool,
            tc.tile_pool(name="stage", bufs=4) as xpool,
            tc.tile_pool(name="fold", bufs=2) as fpool,
            tc.tile_pool(name="psum", bufs=8, space="PSUM") as ppool,
        ):
            # feat chunks by m-block: (2, 2, 4, 4, 4)
            FM = (2, 2, 4, 4, 4)
            FM0 = (0, 2, 4, 8, 12)
            feat_sb = [
                cpool.tile([P, KO, nm * P], f8, name=f"feat_{c}", tag=f"f{c}")
                for c, nm in enumerate(FM)
            ]

            def feat_dma(c):
                nc.sync.dma_start(
                    feat_sb[c], dram_seg(featT, P * KO * (FM0[c] * P), FM[c] * P)
                )

            def feat_slice(kop, m):
                c = next(i for i in range(4, -1, -1) if FM0[i] <= m)
                r = (m - FM0[c]) * P
                return feat_sb[c][:, 2 * kop : 2 * kop + 2, r : r + P]

            # group-maxima and output tiles hold 4 m-blocks each: all their
            # readers/writers are DVE instructions (engine-FIFO ordered), so
            # coarse tags cost nothing and save epilogue semaphore resets
            gmax_sb = [
                apool.tile([P, 4 * NGROUP], f16, name=f"gmax_{g}", tag=f"gm{g}")
                for g in range(MB // 4)
            ]
            # per m: cols 0..23 top-8 values (fp16 via bitcast), 24..47 idx
            out_sb = [
                apool.tile([P, 4, 48], u16, name=f"out_{g}", tag=f"o{g}")
                for g in range(MB // 4)
            ]

            def gmax(m, lo, hi):
                return gmax_sb[m // 4][:, (m % 4) * NGROUP + lo : (m % 4) * NGROUP + hi]

            def oslot(m, lo, hi):
                return out_sb[m // 4][:, m % 4, lo:hi]

            def top8(m, lo, hi, slot):
                vs = oslot(m, slot, slot + 8).bitcast(f16)
                nc.vector.max(out=vs, in_=gmax(m, lo, hi))
                nc.vector.max_index(
                    out=oslot(m, 24 + slot, 24 + slot + 8),
                    in_max=vs,
                    in_values=gmax(m, lo, hi),
                )

            # odd half-tile first (chunk cols 12288:12544 -> gmax[:, 0:128],
            # 128 groups of 2: cols 12288 + j + 128k, k<2).  DMA issue
            # order front-loads the first compute dependencies: odd train
            # tile, feat chunks 0-2, pair-0 train, feat chunks 3-4.
            trodd_sb = spool.tile([P, KO, HT], f8, name="trodd_sb", tag="trodd")
            nc.sync.dma_start(trodd_sb, train_seg(0, HT))
            feat_dma(0)
            feat_dma(1)
            feat_dma(2)
            tr0_sb = spool.tile([P, KO, 2 * TILE_N], f8, name="tr_sb", tag="train")
            nc.sync.dma_start(tr0_sb, train_seg(ODD_SZ, 1024))
            feat_dma(3)
            feat_dma(4)

            for m in range(MB):
                ps = ppool.tile([P, TILE_N], f32, name="ps", tag="ps")
                for kop in range(KO // 2):
                    nc.tensor.matmul(
                        ps[:, :HT],
                        lhsT=feat_slice(kop, m),
                        rhs=trodd_sb[:, 2 * kop : 2 * kop + 2, :],
                        start=(kop == 0),
                        stop=(kop == KO // 2 - 1),
                        perf_mode=DR,
                    )
                s16 = xpool.tile([P, 2 * TILE_N], f16, name="s16", tag="s16")
                nc.scalar.copy(out=s16[:, :HT], in_=ps[:, :HT])
                nc.vector.tensor_tensor(
                    gmax(m, 0, P), s16[:, :128], s16[:, 128:HT], MAX
                )

            # 12 tile pairs (chunk cols 1024p : 1024p+1024 ->
            # gmax[:, 128+128p : 256+128p], groups of 8: 1024p + j + 128k)
            for p in range(NPAIR):
                if p == 0:
                    tr_sb = tr0_sb
                else:
                    tr_sb = spool.tile(
                        [P, KO, 2 * TILE_N], f8, name="tr_sb", tag="train"
                    )
                    nc.sync.dma_start(
                        tr_sb, train_seg(ODD_SZ + p * P * KO * 1024, 1024)
                    )
                for m in range(MB):
                    psA = ppool.tile([P, TILE_N], f32, name="ps", tag="ps")
                    psB = ppool.tile([P, TILE_N], f32, name="ps", tag="ps")
                    for kop in range(KO // 2):
                        for ps, col in ((psA, 0), (psB, TILE_N)):
                            nc.tensor.matmul(
                                ps,
                                lhsT=feat_slice(kop, m),
                                rhs=tr_sb[:, 2 * kop : 2 * kop + 2, col : col + TILE_N],
                                start=(kop == 0),
                                stop=(kop == KO // 2 - 1),
                                perf_mode=DR,
                            )
                    s16 = xpool.tile([P, 2 * TILE_N], f16, name="s16", tag="s16")
                    nc.scalar.copy(out=s16[:, :TILE_N], in_=psA)
                    nc.scalar.copy(out=s16[:, TILE_N:], in_=psB)
                    fa = fpool.tile([P, TILE_N], f16, name="fh", tag="fh")
                    nc.vector.tensor_tensor(fa, s16[:, :TILE_N], s16[:, TILE_N:], MAX)
                    fb = fpool.tile([P, 256], f16, name="fa", tag="fa")
                    nc.vector.tensor_tensor(fb, fa[:, :256], fa[:, 256:], MAX)
                    off = P + p * P
                    nc.vector.tensor_tensor(
                        gmax(m, off, off + P), fb[:, :128], fb[:, 128:], MAX
                    )
                    # staggered top-8 extraction:
                    #   part A (ready after pair 6): spread over pairs 7..9
                    #   part B (ready after pair 9): all at pair 10
                    #   part C (ready after pair 11): per-m at pair 11
                    if (
                        (p == 7 and m < 6)
                        or (p == 8 and 6 <= m < 11)
                        or (p == 9 and 11 <= m)
                    ):
                        top8(m, *PARTS[0], 0)
                    if p == 10:
                        top8(m, *PARTS[1], 8)
                    if p == NPAIR - 1:
                        top8(m, *PARTS[2], 16)

            oo = out_all.ap().rearrange("(g j p) c -> g p j c", p=P, j=4)
            for g in range(MB // 4):
                nc.sync.dma_start(oo[g], out_sb[g])

    nc.compile()
    return nc


def _get_nc():
    if "nc" not in _NC_CACHE:
        _NC_CACHE["nc"] = _build_bass()
    return _NC_CACHE["nc"]


def _group_cols():
    """(NGROUP, 8) chunk-column members per group; -1 = unused slot."""
    cols = np.full((NGROUP, 8), -1, np.int64)
    j = np.arange(128)
    k = np.arange(8)
    for p in range(NPAIR):
        cols[P + p * P : P + (p + 1) * P] = 1024 * p + j[:, None] + 128 * k[None, :]
    cols[0:P, :2] = 12288 + j[:, None] + 128 * np.arange(2)[None, :]
    return cols


_COLS = _group_cols()


def _vote(topv, labels):
    """Reproduce the reference's softmax voting given sorted top sims."""
    x = (topv / np.float32(T)).astype(np.float32)
    e = np.exp(x - x[:, :1], dtype=np.float32)
    s = e.sum(axis=1, keepdims=True, dtype=np.float32)
    w = (e / s).astype(np.float32)
    rows = np.broadcast_to(np.arange(topv.shape[0])[:, None], labels.shape)
    outs = []
    for k in KS:
        p = np.zeros((topv.shape[0], NUM_CLASSES), np.float32)
        np.add.at(p, (rows[:, :k], labels[:, :k]), w[:, :k])
        outs.append(p)
    return outs


def _exact_rows(F, TR, LB, rows, outs):
    """Batched exact recompute of the given rows (reference semantics)."""
    if len(rows) == 0:
        return
    s = (F[rows] @ TR.T).astype(np.float32)
    o = np.argsort(-s.astype(np.float64), axis=1, kind="stable")[:, :200]
    topv = np.take_along_axis(s, o, axis=1).astype(np.float32)
    labs = LB[o].astype(np.int64)
    sub = _vote(topv, labs)
    for i in range(len(KS)):
        outs[i][rows] = sub[i]


def _combine(F, TR, LB, vals, idxs):
    """vals/idxs: (NCORES, B, 24) fp32 / int64 device candidates."""
    # global group ids: slots 0..7 part A, 8..15 part B, 16..23 part C
    base = np.repeat([PARTS[0][0], PARTS[1][0], PARTS[2][0]], 8)
    gl = idxs + base[None, None, :]
    gl = gl + np.arange(NCORES)[:, None, None] * NGROUP
    v = vals.transpose(1, 0, 2).reshape(B, NCORES * 24).astype(np.float32)
    g = gl.transpose(1, 0, 2).reshape(B, NCORES * 24)

    order = np.argsort(-v, axis=1)
    v_s = np.take_along_axis(v, order, axis=1)
    g_s = np.take_along_axis(g, order, axis=1)
    amax = v_s[:, 0]

    # triggers
    trig = v_s[:, NG - 1] >= amax - MARGIN  # NG-th group near window
    # per-(core,part) duplicated group index near the top (fp16 tie)
    iv = idxs.transpose(1, 0, 2).reshape(B, NCORES, 3, 8)
    vv = vals.transpose(1, 0, 2).reshape(B, NCORES, 3, 8).astype(np.float32)
    si = np.sort(iv, axis=3)
    dup = (np.diff(si, axis=3) == 0).any(axis=3) & (
        vv.max(axis=3) >= amax[:, None, None] - MARGIN
    )
    trig |= dup.any(axis=(1, 2))
    # a part's 8th slot near the top (its 9th group may be within window)
    trig |= (vv[:, :, :, 7] >= amax[:, None, None] - MARGIN).any(axis=(1, 2))
    _DEBUG_STATS["trigger_rows"] = int(trig.sum())

    # exact rescore of the top NG groups' member columns
    top_g = g_s[:, :NG]
    core = top_g // NGROUP
    mem = _COLS[top_g % NGROUP]  # (B, NG, 8) chunk cols, -1 pad
    gcol = mem + core[:, :, None] * NCHUNK
    valid = (mem >= 0) & (gcol < N)
    gflat = np.where(valid, gcol, 0).reshape(B, NG * 8)

    exact = np.empty((B, NG * 8), np.float32)
    step = 256
    for b0 in range(0, B, step):
        b1 = min(b0 + step, B)
        exact[b0:b1] = np.einsum(
            "bkd,bd->bk", TR[gflat[b0:b1]], F[b0:b1], optimize=True
        )
    exact = np.where(valid.reshape(B, NG * 8), exact, -np.inf)

    # sort by exact value desc, ties by train index asc (lax.top_k order)
    ordk = np.lexsort((gflat, -exact.astype(np.float64)), axis=1)
    exact_s = np.take_along_axis(exact, ordk, axis=1)[:, :200].astype(np.float32)
    col_s = np.take_along_axis(gflat, ordk, axis=1)[:, :200]

    labels = np.where(exact_s > -np.inf, LB[col_s], 0).astype(np.int64)
    outs = _vote(exact_s, labels)

    _exact_rows(F, TR, LB, np.where(trig)[0], outs)
    return tuple(outs)


def _swz(seg8):
    """(ncols, D) fp8 rows -> device layout [p][ko][n], flattened."""
    return np.ascontiguousarray(
        seg8.T.reshape(KO, P, seg8.shape[0]).transpose(1, 0, 2)
    ).ravel()


def make_in_maps(F, TR):
    TRp = np.zeros((NCORES * NCHUNK, D), F8_DT)
    TRp[:N] = TR.astype(F8_DT)
    F8 = F.astype(F8_DT)
    fb = [(0, 256), (256, 512), (512, 1024), (1024, 1536), (1536, 2048)]
    feat8 = np.concatenate([_swz(F8[a:b]) for a, b in fb])
    in_maps = []
    for c in range(NCORES):
        TRc = TRp[c * NCHUNK : (c + 1) * NCHUNK]
        segs = [_swz(TRc[NPAIR * 1024 : NCHUNK])]
        segs += [_swz(TRc[p * 1024 : (p + 1) * 1024]) for p in range(NPAIR)]
        in_maps.append({"featT": feat8, "trainT": np.concatenate(segs)})
    return in_maps


def kernel(features_rank, train_features, train_labels):
    from concourse.bass_utils import run_bass_kernel_spmd

    F = np.ascontiguousarray(np.asarray(features_rank, dtype=np.float32))
    TR = np.ascontiguousarray(np.asarray(train_features, dtype=np.float32))
    LB = np.asarray(train_labels)

    nc = _get_nc()
    res = run_bass_kernel_spmd(nc, make_in_maps(F, TR), core_ids=list(range(NCORES)))

    raw = np.stack([np.asarray(res.results[c]["gout"]) for c in range(NCORES)])
    vals = raw[:, :, :24].view(np.float16).astype(np.float32)
    idxs = raw[:, :, 24:].astype(np.int64)
    return _combine(F, TR, LB, vals, idxs)
